# revision 12
# baseline (speedup 1.0000x reference)
"""DeepseekV3 MLA attention Bass kernel for 8 Trainium2 NeuronCores.

Sharding: core c -> batch b = c//4, head-group g = c%4 (4 heads each).
On device everything runs in transposed [feature_partition, token_free]
layout; weights are host-prepacked into PE-stationary chunk layout;
RoPE interleave permutation and RMSNorm scale weights are folded into
the weight matrices on host. Matmuls run as float32r (full PE rate,
~1e-4 rel err). Causal structure: 512-wide key tiles whose mask is
entirely <= -1e8 are skipped; output buffers are pre-zeroed by the
runtime so the skipped upper triangle of probs stays exactly 0.

SBUF is ~208KB/partition and Tile pool allocations are static, so the
kernel is split into phases with scoped pools and DRAM intermediates:
  A : q_a/ckv down-projections (+ rmsnorm stats)     -> qa_raw, ckv_raw
  B2: kv up-projection, k_pe rope, V transpose       -> kvT_d, V_d, kpe_d
  B1: q up-projection + q_pe rope                    -> qT_d
  C : per-head flash-style attention + out-projection -> probs_o, outT_o
"""
import numpy as np

B, S, HID = 2, 2048, 2048
NH = 16
QL, KVL = 1536, 512
NOPE, ROPE, VD = 128, 64, 128
QHD = NOPE + ROPE
THETA = 10000.0
EPS = 1e-6
SCALE = QHD ** (-0.5)
HPC = 4            # heads per core
NCORES = 8
P = 128
T4 = 4             # token tiles of 512
TT = 512           # token tile size
NQI = S // P       # 16 query blocks of 128

_cache = {}


def _build(needed, nonzero, nz_index, nnz):
    """Build the Bass program. `needed[qi][kt]`/`nonzero[qi][kt]` are static
    bools over (128-query-block, 512-key-tile); nz_index maps (qi,kt) to a
    slot in the maskNZ input."""
    import sys
    if '/opt/trn_rl_repo' not in sys.path:
        sys.path.insert(0, '/opt/trn_rl_repo')
    import concourse.bacc as bacc
    import concourse.tile as tile
    from concourse import mybir

    f32 = mybir.dt.float32
    f32r = mybir.dt.float32r
    ACT = mybir.ActivationFunctionType
    try:
        from concourse.bass import AxisListType as AX
    except Exception:
        import bass_rust
        AX = bass_rust.AxisListType

    Kt = [max(k for k in range(len(needed[qi])) if needed[qi][k]) + 1
          for qi in range(NQI)]

    nc = bacc.Bacc("TRN2", target_bir_lowering=False, debug=False)

    # ---- I/O ----
    hT = nc.dram_tensor("hT", [HID, S], f32, kind="ExternalInput")
    qaw = nc.dram_tensor("qaw", [P, 12, 16, P], f32, kind="ExternalInput")
    kvaw = nc.dram_tensor("kvaw", [P, 4, 16, P], f32, kind="ExternalInput")
    kvawr = nc.dram_tensor("kvawr", [P, 16, ROPE], f32, kind="ExternalInput")
    qbw = nc.dram_tensor("qbw", [P, 6, 12, P], f32, kind="ExternalInput")
    kvbw = nc.dram_tensor("kvbw", [P, 8, 4, P], f32, kind="ExternalInput")
    ow = nc.dram_tensor("ow", [P, 16, 4, P], f32, kind="ExternalInput")
    cosT2 = nc.dram_tensor("cosT2", [P, S], f32, kind="ExternalInput")
    sinT2 = nc.dram_tensor("sinT2", [P, S], f32, kind="ExternalInput")
    perm = nc.dram_tensor("perm", [P, P], f32, kind="ExternalInput")
    ident = nc.dram_tensor("ident", [P, P], f32, kind="ExternalInput")
    ones_a = nc.dram_tensor("ones_a", [P, 1], f32, kind="ExternalInput")
    ones_b = nc.dram_tensor("ones_b", [1, P], f32, kind="ExternalInput")
    maskNZ = nc.dram_tensor("maskNZ", [max(nnz, 1), P, TT], f32,
                            kind="ExternalInput")
    probs_o = nc.dram_tensor("probs_o", [HPC, S, S], f32, kind="ExternalOutput")
    outT_o = nc.dram_tensor("outT_o", [HID, S], f32, kind="ExternalOutput")
    # DRAM intermediates
    qa_raw = nc.dram_tensor("qa_raw", [QL, S], f32)
    ckv_raw = nc.dram_tensor("ckv_raw", [KVL + ROPE, S], f32)
    qT_d = nc.dram_tensor("qT_d", [6, P, S], f32)
    kvT_d = nc.dram_tensor("kvT_d", [HPC, P, S], f32)
    V_d = nc.dram_tensor("V_d", [HPC, P, 16 * P], f32)
    kpe_d = nc.dram_tensor("kpe_d", [P, S], f32)

    with tile.TileContext(nc) as tc:
        with tc.tile_pool(name="long", bufs=1) as lp:
            ones_a_sb = lp.tile([P, 1], f32r, tag="ones_a")
            nc.sync.dma_start(out=ones_a_sb, in_=ones_a[:, :].bitcast(f32r))
            ones_b_sb = lp.tile([1, P], f32r, tag="ones_b")
            nc.sync.dma_start(out=ones_b_sb, in_=ones_b[:, :].bitcast(f32r))
            perm_sb = lp.tile([P, P], f32r, tag="perm")
            nc.sync.dma_start(out=perm_sb, in_=perm[:, :].bitcast(f32r))
            ident_sb = lp.tile([P, P], f32, tag="ident")
            nc.sync.dma_start(out=ident_sb, in_=ident[:, :])
            eps_sb = lp.tile([1, 1], f32, tag="eps")
            nc.vector.memset(eps_sb, EPS)
            ssq_qa = lp.tile([1, S], f32, tag="ssq_qa")
            nc.vector.memset(ssq_qa, 0.0)
            ssq_ckv = lp.tile([1, S], f32, tag="ssq_ckv")
            nc.vector.memset(ssq_ckv, 0.0)
            rstdb_qa = lp.tile([P, S], f32, tag="rstdb_qa")
            rstdb_ckv = lp.tile([P, S], f32, tag="rstdb_ckv")
            cos_sb = lp.tile([P, S], f32, tag="cos")
            nc.sync.dma_start(out=cos_sb, in_=cosT2[:, :])
            sin_sb = lp.tile([P, S], f32, tag="sin")
            nc.sync.dma_start(out=sin_sb, in_=sinT2[:, :])

            # ================= Phase A: q_a / ckv projections ============
            fts = [("qa", i) for i in range(12)] + \
                  [("ckv", i) for i in range(4)] + [("rope", 0)]
            with tc.tile_pool(name="pa", bufs=1) as pa, \
                 tc.tile_pool(name="pa2", bufs=2) as pa2, \
                 tc.tile_pool(name="pax", bufs=3) as pax, \
                 tc.tile_pool(name="psa", bufs=3, space="PSUM") as psa, \
                 tc.tile_pool(name="psx", bufs=1, space="PSUM") as psx:
                for half in range(2):
                    hsl = slice(half * 1024, (half + 1) * 1024)
                    hT_sb = []
                    for kc in range(16):
                        t = pa.tile([P, 1024], f32r, tag=f"hA{kc}",
                                    name=f"hA{kc}_{half}")
                        nc.sync.dma_start(out=t,
                                          in_=hT[kc * P:(kc + 1) * P, hsl].bitcast(f32r))
                        hT_sb.append(t)
                    for kind, fi in fts:
                        if kind == "qa":
                            wsl = pa2.tile([P, 16, P], f32r, tag="wslA")
                            nc.sync.dma_start(out=wsl, in_=qaw[:, fi].bitcast(f32r))
                            rows, dst, dst_row = P, qa_raw, fi * P
                        elif kind == "ckv":
                            wsl = pa2.tile([P, 16, P], f32r, tag="wslA")
                            nc.sync.dma_start(out=wsl, in_=kvaw[:, fi].bitcast(f32r))
                            rows, dst, dst_row = P, ckv_raw, fi * P
                        else:
                            wsl = pa2.tile([P, 16, ROPE], f32r, tag="wslA")
                            nc.sync.dma_start(out=wsl, in_=kvawr[:, :].bitcast(f32r))
                            rows, dst, dst_row = ROPE, ckv_raw, KVL
                        raw = pa2.tile([P, 1024], f32, tag="rawA")
                        for t2 in range(2):
                            col = half * 1024 + t2 * TT
                            pm = psa.tile([P, TT], f32, tag="pmA")
                            for kc in range(16):
                                nc.tensor.matmul(pm[:rows, :], wsl[:, kc, :rows],
                                                 hT_sb[kc][:, t2 * TT:(t2 + 1) * TT],
                                                 start=(kc == 0), stop=(kc == 15))
                            nc.scalar.activation(out=raw[:rows, t2 * TT:(t2 + 1) * TT],
                                                 in_=pm[:rows, :], func=ACT.Copy)
                            if kind != "rope":
                                x2 = pax.tile([P, TT], f32r, tag="x2")
                                rsl = raw[:, t2 * TT:(t2 + 1) * TT]
                                nc.vector.tensor_mul(out=x2, in0=rsl, in1=rsl)
                                sq = psx.tile([1, TT], f32, tag="sq")
                                nc.tensor.matmul(sq, ones_a_sb, x2,
                                                 start=True, stop=True)
                                tgt = ssq_qa if kind == "qa" else ssq_ckv
                                nc.vector.tensor_add(out=tgt[:, col:col + TT],
                                                     in0=tgt[:, col:col + TT],
                                                     in1=sq)
                        nc.sync.dma_start(out=dst[dst_row:dst_row + rows, hsl],
                                          in_=raw[:rows, :])

                # rstd vectors + broadcast to 128 partitions
                nc.scalar.activation(out=ssq_qa, in_=ssq_qa, func=ACT.Sqrt,
                                     scale=1.0 / QL, bias=eps_sb)
                rstd_qa = pa.tile([1, S], f32r, tag="rstd_qa")
                with nc.allow_low_precision(reason="f32r rounding for PE bcast"):
                    nc.vector.reciprocal(out=rstd_qa, in_=ssq_qa)
                nc.scalar.activation(out=ssq_ckv, in_=ssq_ckv, func=ACT.Sqrt,
                                     scale=1.0 / KVL, bias=eps_sb)
                rstd_ckv = pa.tile([1, S], f32r, tag="rstd_ckv")
                with nc.allow_low_precision(reason="f32r rounding for PE bcast"):
                    nc.vector.reciprocal(out=rstd_ckv, in_=ssq_ckv)
                for t4 in range(T4):
                    sl = slice(t4 * TT, (t4 + 1) * TT)
                    bc = psa.tile([P, TT], f32, tag="pmA")
                    nc.tensor.matmul(bc, ones_b_sb, rstd_qa[:, sl],
                                     start=True, stop=True)
                    nc.vector.tensor_copy(out=rstdb_qa[:, sl], in_=bc)
                    bc2 = psa.tile([P, TT], f32, tag="pmA")
                    nc.tensor.matmul(bc2, ones_b_sb, rstd_ckv[:, sl],
                                     start=True, stop=True)
                    nc.vector.tensor_copy(out=rstdb_ckv[:, sl], in_=bc2)

            # ============ Phase B2: kv up-projection + k_pe rope =========
            with tc.tile_pool(name="pb", bufs=1) as pb, \
                 tc.tile_pool(name="pb2", bufs=2) as pb2, \
                 tc.tile_pool(name="psb", bufs=3, space="PSUM") as psb, \
                 tc.tile_pool(name="psh", bufs=1, space="PSUM") as psh:
                ckv_sb = []
                for kc in range(4):
                    t = pb.tile([P, S], f32r, tag=f"ckv{kc}", name=f"ckv{kc}")
                    nc.sync.dma_start(out=t,
                                      in_=ckv_raw[kc * P:(kc + 1) * P, :].bitcast(f32r))
                    nc.vector.tensor_mul(out=t, in0=t.bitcast(f32), in1=rstdb_ckv)
                    ckv_sb.append(t)
                # k_pe rope, duplicated into both partition halves
                kpe_raw = pb.tile([P, S], f32r, tag="kpe_raw")
                nc.sync.dma_start(out=kpe_raw[:ROPE, :],
                                  in_=ckv_raw[KVL:KVL + ROPE, :].bitcast(f32r))
                nc.sync.dma_start(out=kpe_raw[ROPE:2 * ROPE, :],
                                  in_=ckv_raw[KVL:KVL + ROPE, :].bitcast(f32r))
                a_sb = pb.tile([P, S], f32, tag="ropeA")
                nc.vector.tensor_mul(out=a_sb, in0=kpe_raw.bitcast(f32), in1=cos_sb)
                kpe_b = pb.tile([P, S], f32, tag="kpe_b")
                for t4 in range(T4):
                    sl = slice(t4 * TT, (t4 + 1) * TT)
                    sh = psh.tile([P, TT], f32, tag="shuf")
                    nc.tensor.matmul(sh, perm_sb, kpe_raw[:, sl],
                                     start=True, stop=True)
                    t1 = pb2.tile([P, TT], f32, tag="ropet1")
                    nc.vector.tensor_mul(out=t1, in0=sh, in1=sin_sb[:, sl])
                    nc.vector.tensor_add(out=kpe_b[:, sl], in0=a_sb[:, sl], in1=t1)
                nc.sync.dma_start(out=kpe_d[:, :], in_=kpe_b)

                for ft in range(8):
                    wsl = pb2.tile([P, 4, P], f32r, tag="wslB2")
                    nc.sync.dma_start(out=wsl, in_=kvbw[:, ft].bitcast(f32r))
                    build = pb2.tile([P, S], f32, tag="kvbuild")
                    for t4 in range(T4):
                        sl = slice(t4 * TT, (t4 + 1) * TT)
                        pm = psb.tile([P, TT], f32, tag="pmB")
                        for kc in range(4):
                            nc.tensor.matmul(pm, wsl[:, kc, :], ckv_sb[kc][:, sl],
                                             start=(kc == 0), stop=(kc == 3))
                        nc.scalar.activation(out=build[:, sl], in_=pm, func=ACT.Copy)
                    if ft < 4:
                        nc.sync.dma_start(out=kvT_d[ft], in_=build)
                    else:
                        vb = pb2.tile([P, 16, P], f32, tag="vbuild")
                        for kb in range(16):
                            tp = psh.tile([P, P], f32, tag="shuf")
                            nc.tensor.transpose(tp, build[:, kb * P:(kb + 1) * P],
                                                ident_sb)
                            nc.scalar.activation(out=vb[:, kb, :], in_=tp,
                                                 func=ACT.Copy)
                        nc.sync.dma_start(out=V_d[ft - 4], in_=vb.rearrange("p a b -> p (a b)"))

            # ================= Phase B1: q projection + rope =============
            with tc.tile_pool(name="pc1", bufs=1) as pc1, \
                 tc.tile_pool(name="pc2", bufs=2) as pc2, \
                 tc.tile_pool(name="psc1", bufs=3, space="PSUM") as psc1, \
                 tc.tile_pool(name="psh1", bufs=1, space="PSUM") as psh1:
                wslq = []
                for ft in range(6):
                    w = pc1.tile([P, 12, P], f32r, tag=f"wq{ft}", name=f"wq{ft}")
                    nc.sync.dma_start(out=w, in_=qbw[:, ft].bitcast(f32r))
                    wslq.append(w)
                qb_sb = [pc1.tile([P, S], f32r, tag=f"qb{ft}", name=f"qb{ft}")
                         for ft in range(6)]
                for t4 in range(T4):
                    sl = slice(t4 * TT, (t4 + 1) * TT)
                    qa_sb = []
                    for kc in range(12):
                        t = pc1.tile([P, TT], f32r, tag=f"qa{kc}",
                                     name=f"qa{kc}_{t4}")
                        nc.sync.dma_start(out=t,
                                          in_=qa_raw[kc * P:(kc + 1) * P, sl].bitcast(f32r))
                        nc.vector.tensor_mul(out=t, in0=t.bitcast(f32),
                                             in1=rstdb_qa[:, sl])
                        qa_sb.append(t)
                    for ft in range(6):
                        pm = psc1.tile([P, TT], f32, tag="pmC")
                        for kc in range(12):
                            nc.tensor.matmul(pm, wslq[ft][:, kc, :], qa_sb[kc],
                                             start=(kc == 0), stop=(kc == 11))
                        nc.vector.tensor_copy(out=qb_sb[ft][:, sl], in_=pm)
                # rope on q tiles 4,5 (head pairs), then spill all
                for ft in range(6):
                    if ft >= 4:
                        a2 = pc2.tile([P, S], f32, tag="ropeA2")
                        nc.vector.tensor_mul(out=a2, in0=qb_sb[ft].bitcast(f32),
                                             in1=cos_sb)
                        for t4 in range(T4):
                            sl = slice(t4 * TT, (t4 + 1) * TT)
                            sh = psh1.tile([P, TT], f32, tag="shufq")
                            nc.tensor.matmul(sh, perm_sb, qb_sb[ft][:, sl],
                                             start=True, stop=True)
                            t1 = pc2.tile([P, TT], f32, tag="ropet1q")
                            nc.vector.tensor_mul(out=t1, in0=sh, in1=sin_sb[:, sl])
                            nc.vector.tensor_add(out=qb_sb[ft][:, sl],
                                                 in0=a2[:, sl], in1=t1)
                    nc.sync.dma_start(out=qT_d[ft], in_=qb_sb[ft].bitcast(f32))

            # ========== Phase C: attention + output projection ===========
            with tc.tile_pool(name="pd1", bufs=2) as pd1, \
                 tc.tile_pool(name="pdk", bufs=1) as pdk, \
                 tc.tile_pool(name="pde", bufs=16) as pde, \
                 tc.tile_pool(name="pds", bufs=2) as pds, \
                 tc.tile_pool(name="pda", bufs=1) as pda, \
                 tc.tile_pool(name="pdm", bufs=4) as pdm, \
                 tc.tile_pool(name="pdr", bufs=4) as pdr, \
                 tc.tile_pool(name="pdo", bufs=2) as pdo, \
                 tc.tile_pool(name="pss", bufs=2, space="PSUM") as pss, \
                 tc.tile_pool(name="pst", bufs=2, space="PSUM") as pst, \
                 tc.tile_pool(name="pso", bufs=2, space="PSUM") as pso:
                kpe_c = pdk.tile([P, S], f32r, tag="kpe_c")
                nc.sync.dma_start(out=kpe_c, in_=kpe_d[:, :].bitcast(f32r))
                attn_all = {}
                for h in range(HPC):
                    kvh = pd1.tile([P, S], f32r, tag="kvh")
                    nc.sync.dma_start(out=kvh, in_=kvT_d[h].bitcast(f32r))
                    vh = pd1.tile([P, 16 * P], f32r, tag="vh")
                    nc.sync.dma_start(out=vh, in_=V_d[h].bitcast(f32r))
                    qn = pd1.tile([P, S], f32r, tag="qn")
                    nc.sync.dma_start(out=qn, in_=qT_d[h].bitcast(f32r))
                    qr = pd1.tile([ROPE, S], f32r, tag="qr")
                    r0 = (h % 2) * ROPE
                    nc.sync.dma_start(out=qr,
                                      in_=qT_d[4 + h // 2, r0:r0 + ROPE, :].bitcast(f32r))
                    for qt in range(T4):
                        mtiles = {}
                        for j in range(4):
                            qi = qt * 4 + j
                            for kt in range(Kt[qi]):
                                if nonzero[qi][kt]:
                                    m = pdm.tile([P, TT], f32, tag="mask")
                                    nc.sync.dma_start(
                                        out=m, in_=maskNZ[nz_index[(qi, kt)]])
                                    mtiles[(j, kt)] = m
                        probs_tiles = {}
                        for j in range(4):
                            qi = qt * 4 + j
                            qsl = slice(qi * P, (qi + 1) * P)
                            reds = pdr.tile([P, 16], f32, tag="reds")
                            etiles = []
                            for kt in range(Kt[qi]):
                                ksl = slice(kt * TT, (kt + 1) * TT)
                                ps = pss.tile([P, TT], f32, tag="score")
                                nc.tensor.matmul(ps, qn[:, qsl], kvh[:, ksl],
                                                 start=True, stop=False)
                                nc.tensor.matmul(ps, qr[:, qsl],
                                                 kpe_c[:ROPE, ksl],
                                                 start=False, stop=True)
                                if (j, kt) in mtiles:
                                    nc.vector.tensor_add(out=ps, in0=ps,
                                                         in1=mtiles[(j, kt)])
                                et = pde.tile([P, TT], f32, tag="exp")
                                nc.scalar.activation(out=et, in_=ps, func=ACT.Exp,
                                                     scale=SCALE,
                                                     accum_out=reds[:, kt:kt + 1])
                                etiles.append(et)
                            recip = pdr.tile([P, 1], f32, tag="recip")
                            nc.vector.reduce_sum(out=recip, in_=reds[:, :Kt[qi]],
                                                 axis=AX.X)
                            nc.vector.reciprocal(out=recip, in_=recip)
                            for kt, et in enumerate(etiles):
                                nc.vector.tensor_scalar_mul(out=et, in0=et,
                                                            scalar1=recip)
                                nc.sync.dma_start(
                                    out=probs_o[h, qsl, kt * TT:(kt + 1) * TT],
                                    in_=et)
                            probs_tiles[j] = etiles
                        # attn^T accumulation over key blocks
                        pa_ps = pso.tile([P, TT], f32, tag="attnps")
                        kbmax = 4 * max(Kt[qt * 4 + j] for j in range(4))
                        for kb in range(kbmax):
                            strip = pds.tile([P, TT], f32r, tag="strip")
                            for j in range(4):
                                qi = qt * 4 + j
                                dst = strip[:, j * P:(j + 1) * P]
                                if kb < 4 * Kt[qi]:
                                    src = probs_tiles[j][kb // 4][
                                        :, (kb % 4) * P:(kb % 4 + 1) * P]
                                    tp = pst.tile([P, P], f32, tag="tpP")
                                    nc.tensor.transpose(tp, src, ident_sb)
                                    nc.vector.tensor_copy(out=dst, in_=tp)
                                else:
                                    nc.vector.memset(dst, 0.0)
                            nc.tensor.matmul(pa_ps, vh[:, kb * P:(kb + 1) * P],
                                             strip, start=(kb == 0),
                                             stop=(kb == kbmax - 1))
                        at = pda.tile([P, TT], f32r, tag=f"at{h}_{qt}",
                                      name=f"at{h}_{qt}")
                        nc.vector.tensor_copy(out=at, in_=pa_ps)
                        attn_all[(h, qt)] = at
                # output projection
                for ft in range(16):
                    owsl = pdo.tile([P, 4, P], f32r, tag="wslD")
                    nc.sync.dma_start(out=owsl, in_=ow[:, ft].bitcast(f32r))
                    for qt in range(T4):
                        po = pso.tile([P, TT], f32, tag="attnps")
                        for h in range(HPC):
                            nc.tensor.matmul(po, owsl[:, h, :], attn_all[(h, qt)],
                                             start=(h == 0), stop=(h == 3))
                        ot = pdo.tile([P, TT], f32, tag="outsb")
                        nc.scalar.activation(out=ot, in_=po, func=ACT.Copy)
                        nc.sync.dma_start(
                            out=outT_o[ft * P:(ft + 1) * P,
                                       qt * TT:(qt + 1) * TT], in_=ot)

    nc.compile()
    return nc


def _interleave_perm():
    # row d of interleaved output reads row sigma(d) of the original
    s = np.empty(ROPE, dtype=np.int64)
    s[:ROPE // 2] = np.arange(0, ROPE, 2)
    s[ROPE // 2:] = np.arange(1, ROPE, 2)
    return s


def _pack_w(W, ftiles, kchunks):
    # packed[p, ft, kc, m] = W[ft*128+m, kc*128+p]
    return np.ascontiguousarray(
        W.reshape(ftiles, P, kchunks, P).transpose(3, 0, 2, 1))


def kernel(hidden_states, attention_mask, position_ids,
           q_a_w, q_a_ln_w, q_b_w, kv_a_w, kv_a_ln_w, kv_b_w, o_w):
    import sys
    if '/opt/trn_rl_repo' not in sys.path:
        sys.path.insert(0, '/opt/trn_rl_repo')
    from concourse.bass_utils import run_bass_kernel_spmd

    f4 = np.float32
    hidden_states = np.asarray(hidden_states, dtype=f4)
    mask = np.asarray(attention_mask, dtype=f4)[0, 0]
    position_ids = np.asarray(position_ids)

    # static causal structure from the mask
    needed, nonzero = [], []
    for qi in range(NQI):
        blk = mask[qi * P:(qi + 1) * P]
        nrow, zrow = [], []
        for kt in range(S // TT):
            m = blk[:, kt * TT:(kt + 1) * TT]
            nrow.append(bool((m > -1e8).any()))
            zrow.append(bool((m != 0.0).any()))
        needed.append(nrow)
        nonzero.append(zrow)
    for qi in range(NQI):
        assert any(needed[qi]), "fully-masked query row unsupported"
    nz_index, nz_tiles = {}, []
    for qi in range(NQI):
        last = max(k for k in range(S // TT) if needed[qi][k])
        for kt in range(last + 1):
            if nonzero[qi][kt]:
                nz_index[(qi, kt)] = len(nz_tiles)
                nz_tiles.append(mask[qi * P:(qi + 1) * P, kt * TT:(kt + 1) * TT])
    nnz = len(nz_tiles)
    maskNZ = (np.stack(nz_tiles) if nnz else np.zeros((1, P, TT), f4))
    maskNZ = np.ascontiguousarray(maskNZ, dtype=f4)

    key = ("v2", tuple(tuple(r) for r in needed),
           tuple(tuple(r) for r in nonzero))
    if key not in _cache:
        _cache[key] = _build(needed, nonzero, nz_index, nnz)
    nc = _cache[key]

    # ---- host-side weight prep ----
    sig = _interleave_perm()
    qaw_p = _pack_w(np.asarray(q_a_w, f4), 12, 16)
    kva = np.asarray(kv_a_w, f4).copy()
    kva[KVL:] = kva[KVL:][sig]            # permute rope rows
    kvaw_p = _pack_w(kva[:KVL], 4, 16)
    # rope part: packed[p, kc, m] = Wr[m, kc*128+p]
    kvawr_p = np.ascontiguousarray(
        kva[KVL:].reshape(ROPE, 16, P).transpose(2, 1, 0))

    qb = np.asarray(q_b_w, f4) * np.asarray(q_a_ln_w, f4)[None, :]
    kvb = np.asarray(kv_b_w, f4) * np.asarray(kv_a_ln_w, f4)[None, :]
    ow_full = np.asarray(o_w, f4)

    # rope tables (post-interleave order == standard emb order)
    inv_freq = (1.0 / (THETA ** (np.arange(0, ROPE, 2, dtype=f4) / ROPE))).astype(f4)
    t = np.arange(S, dtype=f4)
    freqs = np.outer(t, inv_freq).astype(f4)
    emb = np.concatenate([freqs, freqs], axis=-1)
    cos_t, sin_t = np.cos(emb).astype(f4), np.sin(emb).astype(f4)
    sgn = np.where(np.arange(ROPE) < ROPE // 2, -1.0, 1.0).astype(f4)

    permM = np.zeros((P, P), f4)
    for d in range(ROPE):
        permM[(d + 32) % ROPE, d] = 1.0
        permM[ROPE + (d + 32) % ROPE, ROPE + d] = 1.0

    in_maps = []
    for c in range(NCORES):
        b, g = c // 4, c % 4
        heads = range(g * HPC, (g + 1) * HPC)
        rows_n, rows_r = [], []
        for h in heads:
            base = h * QHD
            rows_n.extend(range(base, base + NOPE))
            rows_r.extend((base + NOPE + sig).tolist())
        qb_c = qb[rows_n + rows_r]                      # [768, 1536]
        qbw_p = _pack_w(qb_c, 6, 12)
        rows_n, rows_v = [], []
        for h in heads:
            base = h * (NOPE + VD)
            rows_n.extend(range(base, base + NOPE))
            rows_v.extend(range(base + NOPE, base + NOPE + VD))
        kvb_c = kvb[rows_n + rows_v]                    # [1024, 512]
        kvbw_p = _pack_w(kvb_c, 8, 4)
        owT_c = np.ascontiguousarray(
            ow_full[:, g * HPC * VD:(g + 1) * HPC * VD].T)    # [512, 2048]
        # packed[p, ft, kc, m] = owT[kc*128+p, ft*128+m]
        ow_p = np.ascontiguousarray(
            owT_c.reshape(4, P, 16, P).transpose(1, 2, 0, 3))

        cos_b = cos_t[position_ids[b]].T                # [64, 2048]
        sin_b = (sin_t[position_ids[b]] * sgn[None, :]).T
        in_maps.append({
            "hT": np.ascontiguousarray(hidden_states[b].T),
            "qaw": qaw_p, "kvaw": kvaw_p, "kvawr": kvawr_p,
            "qbw": qbw_p, "kvbw": kvbw_p, "ow": ow_p,
            "cosT2": np.ascontiguousarray(np.vstack([cos_b, cos_b])),
            "sinT2": np.ascontiguousarray(np.vstack([sin_b, sin_b])),
            "perm": permM, "ident": np.eye(P, dtype=f4),
            "ones_a": np.ones((P, 1), f4), "ones_b": np.ones((1, P), f4),
            "maskNZ": maskNZ,
        })

    res = run_bass_kernel_spmd(nc, in_maps, list(range(NCORES)))

    out = np.empty((B, S, HID), f4)
    probs = np.empty((B, NH, S, S), f4)
    for b in range(B):
        acc = res.results[b * 4]["outT_o"].astype(f4).copy()
        for g in range(1, 4):
            acc += res.results[b * 4 + g]["outT_o"]
        out[b] = acc.T
        for g in range(4):
            probs[b, g * HPC:(g + 1) * HPC] = res.results[b * 4 + g]["probs_o"]
    return out, probs
